# revision 11
# baseline (speedup 1.0000x reference)
"""GCN 3-layer message passing kernel for Trainium2 (8 NeuronCores).

Sharding: nodes relabeled by (owner core, degree rank); core c owns 12500
consecutive new-ids and segment-sums messages for its own dst nodes. The
full G' table (per-layer transformed features, dinv folded in) is rebuilt
on every core via AllGather each layer; gathers are node-aligned indirect
DMAs from the replicated table. Host does the layer-1 input transform
(x@W1*dinv), the final pooling and the linear head.
"""

import time
import numpy as np

N_NODES = 100000
N_EDGES = 3200000
FEAT = 30
HID = 30
N_GRAPHS = 512
NCORES = 8
NODES_PER_CORE = N_NODES // NCORES  # 12500
P = 128
NTILES = (NODES_PER_CORE + P - 1) // P  # 98 (last tile holds 84 nodes)

_COMPILED = None
_COMPILED_KEY = None


class _Runner:
    """Compile a Bacc kernel once; run it on NCORES cores via PJRT."""

    def __init__(self, nc, n_cores):
        import jax
        import concourse.mybir as mybir
        from concourse.bass2jax import (
            _bass_exec_p, install_neuronx_cc_hook, partition_id_tensor)
        from jax.sharding import Mesh, PartitionSpec, NamedSharding
        from jax.experimental.shard_map import shard_map

        install_neuronx_cc_hook()
        self.jax = jax
        self.n_cores = n_cores
        partition_name = (nc.partition_id_tensor.name
                          if nc.partition_id_tensor else None)
        in_names, out_names, out_avals, zero_outs = [], [], [], []
        for alloc in nc.m.functions[0].allocations:
            if not isinstance(alloc, mybir.MemoryLocationSet):
                continue
            name = alloc.memorylocations[0].name
            if alloc.kind == "ExternalInput":
                if name != partition_name:
                    in_names.append(name)
            elif alloc.kind == "ExternalOutput":
                shape = tuple(alloc.tensor_shape)
                dtype = mybir.dt.np(alloc.dtype)
                out_names.append(name)
                out_avals.append(jax.core.ShapedArray(shape, dtype))
                zero_outs.append(np.zeros(shape, dtype))
        self.in_names, self.out_names, self.zero_outs = (
            in_names, out_names, zero_outs)
        n_params, n_outs = len(in_names), len(out_avals)
        all_in_names = in_names + out_names + (
            [partition_name] if partition_name else [])

        def _body(*args):
            operands = list(args)
            if partition_name is not None:
                operands.append(partition_id_tensor())
            return tuple(_bass_exec_p.bind(
                *operands,
                out_avals=tuple(out_avals),
                in_names=tuple(all_in_names),
                out_names=tuple(out_names),
                lowering_input_output_aliases=(),
                sim_require_finite=True,
                sim_require_nnan=True,
                nc=nc,
            ))

        try:
            devices = jax.devices("axon")[:n_cores]
        except RuntimeError:
            devices = jax.devices()[:n_cores]
        mesh = Mesh(np.asarray(devices), ("core",))
        self.sharding = NamedSharding(mesh, PartitionSpec("core"))
        self.fn = jax.jit(
            shard_map(_body, mesh=mesh,
                      in_specs=(PartitionSpec("core"),) * (n_params + n_outs),
                      out_specs=(PartitionSpec("core"),) * n_outs,
                      check_rep=False),
            keep_unused=True,
        )

    def put_inputs(self, in_maps):
        per_core = [[np.asarray(m[name]) for name in self.in_names]
                    for m in in_maps]
        concat_in = [
            np.concatenate([per_core[c][i] for c in range(self.n_cores)],
                           axis=0)
            for i in range(len(self.in_names))
        ]
        self.dev_in = [self.jax.device_put(a, self.sharding)
                       for a in concat_in]
        self.dev_zo = [self.jax.device_put(z, self.sharding)
                       for z in self._zo()]

    def _zo(self):
        return [np.concatenate([z] * self.n_cores, axis=0)
                for z in self.zero_outs]

    def call(self):
        res = self.fn(*self.dev_in, *self.dev_zo)
        self.jax.block_until_ready(res)
        return res

    def burst(self, burst=10):
        self.call()
        t0 = time.time()
        res = None
        for _ in range(burst):
            res = self.fn(*self.dev_in, *self.dev_zo)
        self.jax.block_until_ready(res)
        return (time.time() - t0) / burst

    def results(self, res):
        out = []
        for c in range(self.n_cores):
            d = {}
            for i, name in enumerate(self.out_names):
                full = np.asarray(res[i])
                sz = full.shape[0] // self.n_cores
                d[name] = full[c * sz:(c + 1) * sz]
            out.append(d)
        return out


def _build_schedule(edge_index):
    """Host preprocessing: relabel nodes, build per-core gather offsets."""
    src = np.asarray(edge_index[0], dtype=np.int64)
    dst = np.asarray(edge_index[1], dtype=np.int64)

    deg = np.bincount(dst, minlength=N_NODES).astype(np.int64) + 1
    dinv = (1.0 / np.sqrt(np.maximum(deg, 1).astype(np.float64))).astype(np.float32)

    # owner core by round-robin over degree rank; within a core nodes are
    # degree-sorted so each 128-node tile has near-uniform degree
    order = np.argsort(-deg, kind="stable")
    perm = np.empty(N_NODES, dtype=np.int64)
    for c in range(NCORES):
        perm[c * NODES_PER_CORE:(c + 1) * NODES_PER_CORE] = order[c::NCORES]
    inv_perm = np.empty(N_NODES, dtype=np.int64)
    inv_perm[perm] = np.arange(N_NODES)

    nsrc = inv_perm[src]
    ndst = inv_perm[dst]

    core_of = ndst // NODES_PER_CORE
    local = ndst % NODES_PER_CORE
    tile_of = local // P
    part_of = local % P

    key_order = np.lexsort((nsrc, part_of, tile_of, core_of))
    cs = core_of[key_order]
    ts = tile_of[key_order]
    ps = part_of[key_order]
    ss = nsrc[key_order]
    grp = (cs * NTILES + ts) * P + ps
    ngrp = NCORES * NTILES * P
    grp_start = np.searchsorted(grp, np.arange(ngrp), side="left")
    grp_end = np.searchsorted(grp, np.arange(ngrp), side="right")
    counts = (grp_end - grp_start).reshape(NCORES, NTILES, P)

    D_t = np.maximum(counts.max(axis=(0, 2)), 1).astype(np.int64)

    tile_base = np.concatenate([[0], np.cumsum(P * D_t)])
    total_slots = int(tile_base[-1])
    offs = np.full((NCORES, total_slots), N_NODES, dtype=np.int32)
    rank = np.arange(len(grp)) - grp_start[grp]
    slot = tile_base[ts] + rank * P + ps
    offs[cs, slot] = ss.astype(np.int32)

    return {
        "perm": perm, "dinv": dinv,
        "D_t": D_t, "tile_base": tile_base, "total_slots": total_slots,
        "offs": offs,
    }


def _build_program(D_t, tile_base, total_slots, zero_bias):
    import concourse.bass as bass
    import concourse.bacc as bacc
    import concourse.mybir as mybir
    from concourse.tile import TileContext
    from concourse.masks import make_identity

    fp32 = mybir.dt.float32
    nc = bacc.Bacc("TRN2", target_bir_lowering=False, debug=False,
                   num_devices=NCORES)

    g1own = nc.dram_tensor("g1own", [NTILES * P, FEAT], fp32, kind="ExternalInput").ap()
    offsets = nc.dram_tensor("offsets", [total_slots], mybir.dt.int32, kind="ExternalInput").ap()
    dinv_in = nc.dram_tensor("dinv", [NTILES * P, 1], fp32, kind="ExternalInput").ap()
    w2 = nc.dram_tensor("w2", [HID, HID], fp32, kind="ExternalInput").ap()
    w3 = nc.dram_tensor("w3", [HID, HID], fp32, kind="ExternalInput").ap()
    bb = nc.dram_tensor("bb", [P, 3, HID], fp32, kind="ExternalInput").ap()
    h3_out = nc.dram_tensor("h3", [NTILES * P, HID], fp32, kind="ExternalOutput").ap()

    bf16 = mybir.dt.bfloat16
    gown = nc.dram_tensor("gown", [NODES_PER_CORE, FEAT], bf16)
    gfull = nc.dram_tensor("gfull", [N_NODES + 1, FEAT], bf16, addr_space="Shared")

    nfull = NODES_PER_CORE // P          # 97 full tiles
    nrem = NODES_PER_CORE - nfull * P    # 84

    with TileContext(nc) as tc:
        with (
            tc.tile_pool(name="const", bufs=1) as cp,
            tc.tile_pool(name="stageA", bufs=1) as stA,
            tc.tile_pool(name="stageB", bufs=1) as stB,
            tc.tile_pool(name="work", bufs=6) as wp,
            tc.tile_pool(name="small", bufs=6) as sp,
            tc.tile_pool(name="psumT", bufs=2, space="PSUM") as ppT,
            tc.tile_pool(name="psumG", bufs=2, space="PSUM") as ppG,
        ):
            ident = cp.tile([P, P], fp32)
            make_identity(nc, ident[:])
            w2t = cp.tile([HID, HID], fp32)
            nc.sync.dma_start(out=w2t[:], in_=w2[:, :])
            w3t = cp.tile([HID, HID], fp32)
            nc.sync.dma_start(out=w3t[:], in_=w3[:, :])
            bbt = cp.tile([P, 3, HID], fp32)
            nc.sync.dma_start(out=bbt[:], in_=bb[:, :, :])
            dinv_t = cp.tile([P, NTILES], fp32)
            nc.sync.dma_start(
                out=dinv_t[:],
                in_=dinv_in[:, 0].rearrange("(t p) -> p t", p=P),
            )
            ncols = total_slots // P
            offs_all = cp.tile([P, ncols], mybir.dt.int32)
            nc.sync.dma_start(
                out=offs_all[:],
                in_=offsets[:].rearrange("(d p) -> p d", p=P),
            )
            zero_row = cp.tile([1, FEAT], bf16)
            nc.vector.memset(zero_row[:], 0.0)
            nc.sync.dma_start(out=gfull[N_NODES:N_NODES + 1, :], in_=zero_row[:])

            stage = stA.tile([P, NTILES, HID], fp32)
            stage2 = stB.tile([P, NTILES, HID], fp32)
            nc.sync.dma_start(
                out=stage[:],
                in_=g1own[:, :].rearrange("(t p) f -> p t f", p=P),
            )

            def publish(st):
                nc.gpsimd.dma_start(
                    out=gown[:nfull * P, :].rearrange("(t p) f -> p t f", p=P),
                    in_=st[:, :nfull, :],
                )
                if nrem:
                    nc.gpsimd.dma_start(
                        out=gown[nfull * P:, :],
                        in_=st[:nrem, nfull, :],
                    )
                tc.strict_bb_all_engine_barrier()
                nc.gpsimd.collective_compute(
                    "AllGather", mybir.AluOpType.bypass,
                    replica_groups=[list(range(NCORES))],
                    ins=[gown[:, :]], outs=[gfull[:N_NODES, :]],
                )
                tc.strict_bb_all_engine_barrier()

            publish(stage)

            cur_stage, nxt_stage = stage, stage2
            for layer in range(3):
                for t in range(NTILES):
                    D = int(D_t[t])
                    cbase = int(tile_base[t]) // P
                    msg = wp.tile([P, D, FEAT], bf16, tag="msg")
                    for j in range(D):
                        nc.gpsimd.indirect_dma_start(
                            out=msg[:, j, :],
                            out_offset=None,
                            in_=gfull[:, :],
                            in_offset=bass.IndirectOffsetOnAxis(
                                ap=offs_all[:, cbase + j:cbase + j + 1],
                                axis=0),
                        )
                    s0 = sp.tile([P, HID], fp32, tag="s0")
                    nc.vector.tensor_reduce(
                        out=s0[:], in_=msg[:].rearrange("p d f -> p f d"),
                        axis=mybir.AxisListType.X, op=mybir.AluOpType.add,
                    )
                    s1 = sp.tile([P, HID], fp32, tag="s1")
                    nc.vector.tensor_add(
                        out=s1[:], in0=cur_stage[:, t, :], in1=s0[:])
                    h = sp.tile([P, HID], fp32, tag="h")
                    if zero_bias:
                        nc.scalar.activation(
                            h[:], s1[:], mybir.ActivationFunctionType.Relu,
                            bias=0.0, scale=dinv_t[:, t:t + 1])
                    else:
                        s2 = sp.tile([P, HID], fp32, tag="s2")
                        nc.vector.scalar_tensor_tensor(
                            out=s2[:], in0=s1[:], scalar=dinv_t[:, t:t + 1],
                            in1=bbt[:, layer, :],
                            op0=mybir.AluOpType.mult, op1=mybir.AluOpType.add,
                        )
                        nc.scalar.activation(
                            h[:], s2[:], mybir.ActivationFunctionType.Relu)
                    if layer < 2:
                        ht_ps = ppT.tile([HID, P], fp32, tag="tps")
                        nc.tensor.transpose(out=ht_ps[:], in_=h[:],
                                            identity=ident[:])
                        ht = sp.tile([HID, P], fp32, tag="ht")
                        nc.vector.tensor_copy(out=ht[:], in_=ht_ps[:])
                        g_ps = ppG.tile([P, HID], fp32, tag="gps")
                        wmat = w2t if layer == 0 else w3t
                        nc.tensor.matmul(out=g_ps[:], lhsT=ht[:], rhs=wmat[:],
                                         start=True, stop=True)
                        nc.vector.tensor_scalar_mul(
                            out=nxt_stage[:, t, :], in0=g_ps[:],
                            scalar1=dinv_t[:, t:t + 1])
                    else:
                        nc.vector.tensor_copy(out=nxt_stage[:, t, :], in_=h[:])
                if layer < 2:
                    publish(nxt_stage)
                cur_stage, nxt_stage = nxt_stage, cur_stage

            nc.sync.dma_start(
                out=h3_out[:, :].rearrange("(t p) f -> p t f", p=P),
                in_=cur_stage[:],
            )

    nc.compile()
    return nc


def kernel(x, edge_index, batch_ids, W1, b1, W2, b2, W3, b3, lin_W, lin_b):
    global _COMPILED, _COMPILED_KEY
    x = np.asarray(x, dtype=np.float32)
    edge_index = np.asarray(edge_index)
    batch_ids = np.asarray(batch_ids)
    W1 = np.asarray(W1, np.float32); b1 = np.asarray(b1, np.float32)
    W2 = np.asarray(W2, np.float32); b2 = np.asarray(b2, np.float32)
    W3 = np.asarray(W3, np.float32); b3 = np.asarray(b3, np.float32)
    lin_W = np.asarray(lin_W, np.float32); lin_b = np.asarray(lin_b, np.float32)

    sched = _build_schedule(edge_index)
    perm, dinv = sched["perm"], sched["dinv"]

    zero_bias = (not b1.any()) and (not b2.any()) and (not b3.any())
    key = (sched["D_t"].tobytes(), zero_bias)
    if _COMPILED is None or _COMPILED_KEY != key:
        nc = _build_program(sched["D_t"], sched["tile_base"],
                            sched["total_slots"], zero_bias)
        _COMPILED = _Runner(nc, NCORES)
        _COMPILED_KEY = key
    r = _COMPILED

    g1 = (x @ W1) * dinv[:, None]
    g1p = g1[perm]
    dinvp = dinv[perm]

    pad_nodes = NTILES * P
    bbc = np.stack([
        np.broadcast_to(b1, (P, HID)),
        np.broadcast_to(b2, (P, HID)),
        np.broadcast_to(b3, (P, HID)),
    ], axis=1).astype(np.float32)  # [P, 3, HID]
    in_maps = []
    for c in range(NCORES):
        lo, hi = c * NODES_PER_CORE, (c + 1) * NODES_PER_CORE
        g1own = np.zeros((pad_nodes, FEAT), np.float32)
        g1own[:NODES_PER_CORE] = g1p[lo:hi]
        dv = np.zeros((pad_nodes, 1), np.float32)
        dv[:NODES_PER_CORE, 0] = dinvp[lo:hi]
        in_maps.append({
            "g1own": g1own,
            "offsets": sched["offs"][c],
            "dinv": dv,
            "w2": W2, "w3": W3, "bb": bbc,
        })

    r.put_inputs(in_maps)
    res = r.call()
    results = r.results(res)

    h3p = np.concatenate(
        [results[c]["h3"][:NODES_PER_CORE] for c in range(NCORES)], axis=0)
    h3 = np.empty_like(h3p)
    h3[perm] = h3p
    pooled = np.zeros((N_GRAPHS, HID), np.float32)
    np.add.at(pooled, batch_ids.astype(np.int64), h3)
    return pooled @ lin_W + lin_b
